# revision 51
# baseline (speedup 1.0000x reference)
"""Trainium2 Bass kernel for nn_Block_87428354277599 (sinkhorn-attention transformer block).

Self-contained: hardcodes shapes/sharding. kernel(**inputs) -> (2, 2048, 384) f32.

Sharding (8 cores, SPMD):
- 12 (batch, head) units padded to 16 slots: every core runs 2 attention slots
  (cores 4-7's slot 1 gets zero weights; its junk output is never consumed).
- LN1/LN2 are folded into the QKV / MLP matmuls via host-precomputed weight folds
  plus rank-1 corrections (mu and t-column terms) accumulated on the PE.
- Sinkhorn on the row-softmaxed causal attention == multiplicative matrix scaling
  of S = exp(P). S-1 is lower-triangular, so only the lower triangle (S' = S-1)
  is stored SBUF-resident in both layouts (S' f32, S'^T bf16); the all-ones part
  of S becomes global-sum corrections (kept f32). All matvecs run on the PE.
- y^T slices are exchanged with one AllToAll (each sender duplicates its slices
  into both batch shard groups; receivers mask the wrong batch via zeroed halves
  of the duplicated proj weights). proj+LN2+MLP run row-sharded (512 rows/core).
"""

import numpy as np
import ml_dtypes

import concourse.bacc as bacc
import concourse.mybir as mybir
from concourse.tile import TileContext
from concourse.bass_utils import run_bass_kernel_spmd

F32 = mybir.dt.float32
BF16 = mybir.dt.bfloat16
F32R = mybir.dt.float32r
AF = mybir.ActivationFunctionType
ALU = mybir.AluOpType
AXX = mybir.AxisListType.X

B, T, C, H, HD = 2, 2048, 384, 6, 64
CP1 = C + 1
N_CORES = 8
NT = T // 128  # 16
EPS = 1e-5
UNITS = [(u // H, u % H) for u in range(2 * H)]  # 12 real units
CORE_UNITS = {0: [0, 1], 1: [2, 3], 2: [4, 5], 3: [6, 7], 4: [8], 5: [9], 6: [10], 7: [11]}
UNIT_SLOT = {}
for _c, _us in CORE_UNITS.items():
    for _s, _u in enumerate(_us):
        UNIT_SLOT[_u] = (_c, _s)

_COMPILED = {}


def build_program():
    nc = bacc.Bacc(trn_type="TRN2", num_devices=N_CORES)

    def _mm(out, lhsT, rhs, start, stop):
        nc.tensor.matmul(out, lhsT, rhs, start=start, stop=stop)

    _mmb = _mm

    def din(name, shape, dt=F32):
        return nc.dram_tensor(name, list(shape), dt, kind="ExternalInput")

    xT_d = din("xT", (C, T), F32R)
    wqk_d = din("wqk", (2, 3, 128, 128), F32R)
    wv_d = din("wv", (3, 128, 128), F32R)
    r1qk_d = din("r1qk", (1, 512), F32R)
    r1v_d = din("r1v", (1, 256), F32R)
    c1qkr_d = din("c1qkr", (1, 256), F32R)
    c1vr_d = din("c1vr", (1, 128), F32R)
    ident_d = din("ident", (128, 128))
    onesc_d = din("onesc", (128, 1), F32R)
    onesr_d = din("onesr", (1, 128), F32R)
    tcol_d = din("tcol", (128, 1))
    sbias_d = din("sbias", (1, 2))
    epsc_d = din("epsc", (128, 1))
    tmlt_d = din("tmlt", (128, NT))
    wproj_d = din("wproj", (H, 3, 128, 128), F32R)
    bproj_d = din("bproj", (128, 3))
    wf_d = din("wf", (12, 3, 128, 128), F32R)
    nwft_d = din("nwft", (1, 1536), F32R)
    ns2f_d = din("ns2f", (1, 1536), F32R)
    c2b_d = din("c2b", (128, 12))
    wf2_d = din("wf2", (3, 12, 128, 128), F32R)
    bfc2_d = din("bfc2", (128, 3))
    out_d = nc.dram_tensor("oT", [C, 512], F32, kind="ExternalOutput")

    with TileContext(nc) as tc, nc.allow_low_precision(reason="f32r-typed intermediates (same bits as f32)"):
        with (
            tc.tile_pool(name="const", bufs=1) as cpool,
            tc.tile_pool(name="dram", bufs=1, space="DRAM") as dpool,
            tc.tile_pool(name="ps_wide", bufs=1, space="PSUM") as ppw,
            tc.tile_pool(name="ps_mm", bufs=2, space="PSUM") as ppm,
            tc.tile_pool(name="ps_tr", bufs=2, space="PSUM") as ppt,
            tc.tile_pool(name="qk", bufs=1) as qkp,
        ):
            a2a_in = dpool.tile([8, 128, 512], F32, name="a2a_in")
            a2a_out = dpool.tile([8, 128, 512], F32, name="a2a_out")
            bounce = [dpool.tile([1, T], F32R, name=f"bounce{s}") for s in range(2)]
            bnc_pview = [bounce[s][:, :].rearrange("a (f p) -> (a p) f", p=128) for s in range(2)]

            ident = cpool.tile([128, 128], F32, tag="ident", name="ident")
            onesc = cpool.tile([128, 1], F32R, tag="onesc", name="onesc")
            onesr = cpool.tile([1, 128], F32R, tag="onesr", name="onesr")
            tcol = cpool.tile([128, 1], F32, tag="tcol", name="tcol")
            sbias = cpool.tile([1, 2], F32, tag="sbias", name="sbias")
            epsc = cpool.tile([128, 1], F32, tag="epsc", name="epsc")
            nc.sync.dma_start(out=ident[:, :], in_=ident_d[:, :])
            nc.sync.dma_start(out=onesc[:, :], in_=onesc_d[:, :])
            nc.sync.dma_start(out=onesr[:, :], in_=onesr_d[:, :])
            nc.sync.dma_start(out=tcol[:, :], in_=tcol_d[:, :])
            nc.sync.dma_start(out=sbias[:, :], in_=sbias_d[:, :])
            nc.sync.dma_start(out=epsc[:, :], in_=epsc_d[:, :])
            identr = cpool.tile([128, 128], F32R, tag="identr", name="identr")
            nc.scalar.copy(identr[:, :], ident[:, :])
            ident16 = cpool.tile([128, 128], BF16, tag="ident16", name="ident16")
            nc.scalar.copy(ident16[:, :], ident[:, :])
            onescf = cpool.tile([128, 1], F32, tag="onescf", name="onescf")
            onesrf = cpool.tile([1, 128], F32, tag="onesrf", name="onesrf")
            nc.scalar.copy(onescf[:, :], onesc[:, :])
            nc.scalar.copy(onesrf[:, :], onesr[:, :])
            tmlt = cpool.tile([128, NT], F32, tag="tmlt", name="tmlt")
            nc.sync.dma_start(out=tmlt[:, :], in_=tmlt_d[:, :])

            # persistent per-slot activations (base-partition-0 tiles)
            qT = [qkp.tile([64, T], BF16, tag=f"qT{s}", name=f"qT{s}") for s in range(2)]
            kT = [qkp.tile([64, T], BF16, tag=f"kT{s}", name=f"kT{s}") for s in range(2)]
            vrow = [qkp.tile([128, NT * 64], BF16, tag=f"vrow{s}", name=f"vrow{s}") for s in range(2)]

            # ---------------- phase 1+2: stats + QKV (xt-scoped) ----------------
            with tc.tile_pool(name="xt", bufs=1) as xp:
                xT = [xp.tile([128, T], F32R, tag=f"xt{kc}", name=f"xt{kc}") for kc in range(3)]
                for c4 in range(4):
                    for kc in range(3):
                        nc.sync.dma_start(out=xT[kc][:, c4 * 512:(c4 + 1) * 512],
                                          in_=xT_d[kc * 128:(kc + 1) * 128, c4 * 512:(c4 + 1) * 512])
                wqk = [[xp.tile([128, 128], F32R, tag=f"wqk{s}{kc}", name=f"wqk{s}{kc}") for kc in range(3)] for s in range(2)]
                wv = [xp.tile([128, 128], F32R, tag=f"wv{kc}", name=f"wv{kc}") for kc in range(3)]
                r1qk = xp.tile([1, 512], F32R, tag="r1qk", name="r1qk")
                r1v = xp.tile([1, 256], F32R, tag="r1v", name="r1v")
                c1qkr = xp.tile([1, 256], F32R, tag="c1qkr", name="c1qkr")
                c1vr = xp.tile([1, 128], F32R, tag="c1vr", name="c1vr")
                for s in range(2):
                    for kc in range(3):
                        nc.sync.dma_start(out=wqk[s][kc][:, :], in_=wqk_d[s, kc, :, :])
                for kc in range(3):
                    nc.sync.dma_start(out=wv[kc][:, :], in_=wv_d[kc, :, :])
                nc.sync.dma_start(out=r1qk[:, :], in_=r1qk_d[:, :])
                nc.sync.dma_start(out=r1v[:, :], in_=r1v_d[:, :])
                nc.sync.dma_start(out=c1qkr[:, :], in_=c1qkr_d[:, :])
                nc.sync.dma_start(out=c1vr[:, :], in_=c1vr_d[:, :])

                # ---- stats (per 512-token chunk for pipelining) ----
                mu_row = xp.tile([1, T], F32R, tag="mu_row", name="mu_row")
                msq_row = xp.tile([1, T], F32, tag="msq_row", name="msq_row")
                std_row = xp.tile([1, T], F32R, tag="std_row", name="std_row")
                rstdf = xp.tile([1, T], F32, tag="rstdf", name="rstdf")
                rstd_row = xp.tile([1, T], F32R, tag="rstd_row", name="rstd_row")
                bneg_row = xp.tile([1, T], F32R, tag="bneg_row", name="bneg_row")
                rstd_bc = xp.tile([128, T], F32, tag="rstd_bc", name="rstd_bc")
                wide = ppw.tile([128, T], F32, tag="wide", name="wide")
                for c4 in range(4):
                    sl = slice(c4 * 512, (c4 + 1) * 512)
                    for kc in range(3):
                        _mm(wide[0:1, sl], onesc[:, :], xT[kc][:, sl],
                            start=(kc == 0), stop=(kc == 2))
                    nc.scalar.activation(mu_row[0:1, sl], wide[0:1, sl],
                                         AF.Identity, bias=sbias[0:1, 0:1], scale=1.0 / CP1)
                    ps = ppm.tile([1, 512], F32, tag="mm", name="mm")
                    for kc in range(3):
                        sq = xp.tile([128, 512], F32R, tag=f"scr{kc % 2}", name="scr")
                        nc.vector.tensor_tensor(sq[:, :], xT[kc][:, sl], xT[kc][:, sl], ALU.mult)
                        _mm(ps[0:1, :], onesc[:, :], sq[:, :], start=(kc == 0), stop=(kc == 2))
                    nc.scalar.activation(msq_row[0:1, sl], ps[0:1, :],
                                         AF.Identity, bias=sbias[0:1, 1:2], scale=1.0 / CP1)
                    nc.vector.tensor_tensor(std_row[0:1, sl], mu_row[0:1, sl], mu_row[0:1, sl], ALU.mult)
                    nc.vector.tensor_tensor(std_row[0:1, sl], msq_row[0:1, sl], std_row[0:1, sl], ALU.subtract)
                    nc.scalar.activation(std_row[0:1, sl], std_row[0:1, sl], AF.Sqrt, bias=epsc[0:1, 0:1])
                    nc.vector.reciprocal_approx_fast(out=rstdf[0:1, sl], in_=std_row[0:1, sl].bitcast(F32))
                    nc.vector.tensor_copy(rstd_row[0:1, sl], rstdf[0:1, sl])
                    nc.vector.tensor_scalar(bneg_row[0:1, sl], mu_row[0:1, sl], tcol[0:1, 0:1],
                                            None, ALU.subtract)
                    ps2 = ppm.tile([128, 512], F32, tag="mm", name="mm")
                    _mm(ps2[:, :], onesr[:, :], rstd_row[0:1, sl], start=True, stop=True)
                    nc.scalar.copy(rstd_bc[:, sl], ps2[:, :])

                # ---- QKV matmuls: q|k packed 128-wide, bf16 staging, DMA split ----
                v_c = xp.tile([128, T], F32R, tag="v_c", name="v_c")
                qk_cb = [xp.tile([128, T], BF16, tag=f"qk_cb{s}", name=f"qk_cb{s}") for s in range(2)]

                def qkv_mat(dst, lhsT_chunks, r1_trow, r1_s1, c1row):
                    for c4 in range(4):
                        sl = slice(c4 * 512, (c4 + 1) * 512)
                        ps = ppm.tile([128, 512], F32, tag="mm", name="mm")
                        for kc in range(3):
                            _mm(ps[:, :], lhsT_chunks[kc][:, :], xT[kc][:, sl],
                                start=(kc == 0), stop=False)
                        _mm(ps[:, :], r1_trow, bneg_row[0:1, sl], start=False, stop=False)
                        _mm(ps[:, :], r1_s1, mu_row[0:1, sl], start=False, stop=False)
                        # + c1 (x-independent bias) pre-divided by rstd: c1 (x) std
                        _mm(ps[:, :], c1row, std_row[0:1, sl], start=False, stop=True)
                        nc.vector.tensor_tensor(dst[:, sl], ps[:, :], rstd_bc[:, sl], ALU.mult)

                for s in range(2):
                    b0 = 2 * s * 128
                    qkv_mat(qk_cb[s], wqk[s], r1qk[0:1, b0:b0 + 128],
                            r1qk[0:1, b0 + 128:b0 + 256], c1qkr[0:1, s * 128:(s + 1) * 128])
                qkv_mat(v_c, wv, r1v[0:1, 0:128], r1v[0:1, 128:256], c1vr[0:1, 0:128])
                for s in range(2):
                    nc.sync.dma_start(out=qT[s][:, :], in_=qk_cb[s][0:64, :])
                    nc.sync.dma_start(out=kT[s][:, :], in_=qk_cb[s][64:128, :])

                # v -> row-major bf16 via PE transposes
                vA = xp.tile([64, T], F32R, tag="vA", name="vA")
                vB = xp.tile([64, T], F32R, tag="vB", name="vB")
                nc.sync.dma_start(out=vA[:, :], in_=v_c[0:64, :])
                nc.sync.dma_start(out=vB[:, :], in_=v_c[64:128, :])
                for s, vsrc in ((0, vA), (1, vB)):
                    for g0 in range(0, NT, 4):
                        tr = ppt.tile([128, 512], F32R, tag="tr", name="tr")
                        for gi in range(4):
                            jt = g0 + gi
                            nc.tensor.transpose(tr[:, gi * 128:gi * 128 + 64],
                                                vsrc[:, jt * 128:(jt + 1) * 128], identr[0:64, 0:64])
                        for gi in range(4):
                            nc.vector.tensor_copy(vrow[s][:, (g0 + gi) * 64:(g0 + gi + 1) * 64],
                                                  tr[:, gi * 128:gi * 128 + 64])

            # ------- phase 3: attention, both slots interleaved (bf16 triangles) -------
            with (
                tc.tile_pool(name="sp", bufs=1) as spp,
                tc.tile_pool(name="spt", bufs=1) as sptp,
                tc.tile_pool(name="att_misc", bufs=1) as amp,
            ):
                sp = [[spp.tile([128, (it + 1) * 128], BF16, tag=f"sp{s}_{it}", name=f"sp{s}_{it}")
                       for it in range(NT)] for s in range(2)]
                spt = [[sptp.tile([128, (NT - jt) * 128], BF16, tag=f"spt{s}_{jt}", name=f"spt{s}_{jt}")
                        for jt in range(NT)] for s in range(2)]
                e = [[spt[s][NT - 1 - it] for it in range(NT)] for s in range(2)]  # aliases

                zall = [amp.tile([128, NT], F32, tag=f"zall{s}", name=f"zall{s}") for s in range(2)]
                rz = [amp.tile([128, NT], F32, tag=f"rz{s}", name=f"rz{s}") for s in range(2)]
                ssum = [amp.tile([128, NT], F32, tag=f"ssum{s}", name=f"ssum{s}") for s in range(2)]
                apf = [amp.tile([128, NT], F32, tag=f"apf{s}", name=f"apf{s}") for s in range(2)]
                bpf = [amp.tile([128, NT], F32, tag=f"bpf{s}", name=f"bpf{s}") for s in range(2)]
                a16 = [amp.tile([128, NT], BF16, tag=f"a16{s}", name=f"a16{s}") for s in range(2)]
                b16 = [amp.tile([128, NT], BF16, tag=f"b16{s}", name=f"b16{s}") for s in range(2)]
                row_sb = [amp.tile([1, T], F32R, tag=f"row_sb{s}", name=f"row_sb{s}") for s in range(2)]

                # ---- QK^T + exp(qk/8), causal-masked; z via one DVE row reduce ----
                for it in range(NT):
                    L = (it + 1) * 128
                    d0 = it * 128
                    nch = (L + 511) // 512
                    for s in range(2):
                        for c4 in range(nch):
                            lo, hi = c4 * 512, min(L, (c4 + 1) * 512)
                            ps = ppm.tile([128, 512], F32, tag="mm", name="mm")
                            _mm(ps[:, 0:hi - lo], qT[s][:, d0:d0 + 128], kT[s][:, lo:hi],
                                start=True, stop=True)
                            nc.scalar.activation(e[s][it][:, lo:hi], ps[:, 0:hi - lo],
                                                 AF.Exp, scale=0.125)
                        nc.gpsimd.affine_select(out=e[s][it][:, d0:L], in_=e[s][it][:, d0:L],
                                                compare_op=ALU.is_ge, fill=0.0, base=0,
                                                pattern=[[-1, 128]], channel_multiplier=1)
                        nc.vector.tensor_reduce(zall[s][:, it:it + 1], e[s][it][:, 0:L],
                                                axis=AXX, op=ALU.add)
                for s in range(2):
                    nc.vector.reciprocal_approx_fast(out=rz[s][:, :], in_=zall[s][:, :])

                # ---- S' = exp(att)-1; row sums accumulate for free; transposes ride
                # the PE as soon as their source tiles are ready ----
                for it in range(NT):
                    L = (it + 1) * 128
                    for s in range(2):
                        nc.scalar.activation(sp[s][it][:, :], e[s][it][:, 0:L], AF.Exp,
                                             scale=rz[s][:, it:it + 1],
                                             accum_out=ssum[s][:, it:it + 1])
                        nc.vector.tensor_scalar(sp[s][it][:, :], sp[s][it][:, :], -1.0,
                                                None, ALU.add)
                # spt via DRAM-staged XBAR DMA transposes: stage sp tiles to DRAM,
                # read back transposed. Keeps PE/Scalar/Vector free in this stretch.
                spd = [[dpool.tile([128, (it + 1) * 128], BF16, name=f"spd{s}_{it}")
                        for it in range(NT)] for s in range(2)]
                for it in range(NT):
                    for s in range(2):
                        nc.sync.dma_start(out=spd[s][it][:, :], in_=sp[s][it][:, :])
                        for jt in range(it + 1):
                            nc.sync.dma_start_transpose(
                                out=spt[s][jt][:, (it - jt) * 128:(it - jt + 1) * 128],
                                in_=spd[s][it][:, jt * 128:(jt + 1) * 128])
                # first sinkhorn u-update is free: a1 = 1/(T*(T - L + rowsum(exp)))
                for s in range(2):
                    nc.vector.scalar_tensor_tensor(apf[s][:, :], ssum[s][:, :], float(T),
                                                   tmlt[:, :], ALU.mult, ALU.add)
                    nc.vector.reciprocal_approx_fast(out=apf[s][:, :], in_=apf[s][:, :])
                    nc.vector.tensor_copy(a16[s][:, :], apf[s][:, :])

                def gsum_col(src_p, tag):
                    red = amp.tile([128, 1], F32, tag=f"red{tag}", name=f"red{tag}")
                    nc.vector.tensor_reduce(red[:, :], src_p[:, :], axis=AXX, op=ALU.add)
                    ps1 = ppm.tile([1, 512], F32, tag="mm", name="mm")
                    _mm(ps1[0:1, 0:1], onescf[:, :], red[:, :], start=True, stop=True)
                    ssb = amp.tile([1, 1], F32, tag=f"ssb{tag}", name=f"ssb{tag}")
                    nc.scalar.copy(ssb[0:1, :], ps1[0:1, 0:1])
                    psb = ppm.tile([128, 512], F32, tag="mm", name="mm")
                    _mm(psb[:, 0:1], onesrf[:, :], ssb[0:1, 0:1], start=True, stop=True)
                    bc = amp.tile([128, 1], F32, tag=f"bc{tag}", name=f"bc{tag}")
                    nc.scalar.copy(bc[:, :], psb[:, 0:1])
                    return bc

                # ---- sinkhorn: a1 done; now b1, (a2, b2), (a3, b3) ----
                wide = ppw.tile([128, T], F32, tag="wide", name="wide")
                for itr in range(3):
                    # v-update: b = 1/(T*(sum(a) + S'^T a)), S'^T a via sp row-tiles
                    Acol = [gsum_col(apf[s], f"a{s}") for s in range(2)]
                    for s in range(2):
                        for it in range(NT):
                            L = (it + 1) * 128
                            for c4 in range((L + 511) // 512):
                                lo, hi = c4 * 512, min(L, (c4 + 1) * 512)
                                _mm(wide[32 * s:32 * s + 1, lo:hi], a16[s][:, it:it + 1], sp[s][it][:, lo:hi],
                                    start=(it == c4 * 4), stop=(it == NT - 1))
                        nc.scalar.copy(row_sb[s][0:1, 0:1024], wide[32 * s:32 * s + 1, 0:1024])
                        nc.vector.tensor_copy(row_sb[s][0:1, 1024:T], wide[32 * s:32 * s + 1, 1024:T])
                        nc.sync.dma_start(out=bounce[s][:, :], in_=row_sb[s][0:1, :])
                        nc.sync.dma_start(out=bpf[s][:, :].bitcast(F32R), in_=bnc_pview[s])
                        nc.vector.tensor_scalar(bpf[s][:, :], bpf[s][:, :], Acol[s][:, 0:1],
                                                float(T), ALU.add, ALU.mult)
                        nc.vector.reciprocal_approx_fast(out=bpf[s][:, :], in_=bpf[s][:, :])
                        nc.vector.tensor_copy(b16[s][:, :], bpf[s][:, :])
                    if itr == 2:
                        break
                    # u-update: a = 1/(T*(sum(b) + S' b)), S' b via spt col-tiles
                    Bcol = [gsum_col(bpf[s], f"b{s}") for s in range(2)]
                    for s in range(2):
                        for jt in range(NT):
                            j0 = jt * 128
                            for c4 in range(4):
                                lo, hi = c4 * 512, (c4 + 1) * 512
                                if hi <= j0:
                                    continue
                                slo = max(lo, j0)
                                _mmb(wide[32 * s:32 * s + 1, slo:hi], b16[s][:, jt:jt + 1],
                                     spt[s][jt][:, slo - j0:hi - j0],
                                     start=(jt == 0), stop=(jt == min(NT - 1, 4 * c4 + 3)))
                        nc.scalar.copy(row_sb[s][0:1, 0:1024], wide[32 * s:32 * s + 1, 0:1024])
                        nc.vector.tensor_copy(row_sb[s][0:1, 1024:T], wide[32 * s:32 * s + 1, 1024:T])
                        nc.sync.dma_start(out=bounce[s][:, :], in_=row_sb[s][0:1, :])
                        nc.sync.dma_start(out=apf[s][:, :].bitcast(F32R), in_=bnc_pview[s])
                        nc.vector.tensor_scalar(apf[s][:, :], apf[s][:, :], Bcol[s][:, 0:1],
                                                float(T), ALU.add, ALU.mult)
                        nc.vector.reciprocal_approx_fast(out=apf[s][:, :], in_=apf[s][:, :])
                        nc.vector.tensor_copy(a16[s][:, :], apf[s][:, :])

                # ---- y^T = T*a ∘ (S' @ (b∘V) + colsum(b∘V)) ----
                for s in range(2):
                    nc.sync.dma_start(out=bnc_pview[s], in_=apf[s][:, :].bitcast(F32R))
                    nc.sync.dma_start(out=row_sb[s][0:1, :], in_=bounce[s][:, :])
                for s in range(2):
                    yps = wide[64:128, :]
                    wcps = ppm.tile([128, 512], F32, tag="mm", name="mm")
                    for jt in range(NT):
                        j0 = jt * 128
                        bv = amp.tile([128, 64], F32, tag=f"bv{s}_{jt % 2}", name=f"bv{s}")
                        nc.vector.tensor_scalar(bv[:, :], vrow[s][:, jt * 64:(jt + 1) * 64],
                                                bpf[s][:, jt:jt + 1], None, ALU.mult)
                        bvh = amp.tile([128, 64], BF16, tag=f"bvh{s}_{jt % 2}", name=f"bvh{s}")
                        nc.vector.tensor_copy(bvh[:, :], bv[:, :])
                        for c4 in range(4):
                            lo, hi = c4 * 512, (c4 + 1) * 512
                            if hi <= j0:
                                continue
                            slo = max(lo, j0)
                            _mmb(yps[:, slo:hi], bvh[:, :], spt[s][jt][:, slo - j0:hi - j0],
                                 start=(jt == 0), stop=(jt == min(NT - 1, 4 * c4 + 3)))
                        _mm(wcps[0:1, 0:64], onescf[:, :], bv[:, :],
                            start=(jt == 0), stop=(jt == NT - 1))
                    wrow = amp.tile([1, 64], F32, tag=f"wrow{s}", name=f"wrow{s}")
                    nc.scalar.copy(wrow[0:1, :], wcps[0:1, 0:64])
                    wtp = ppm.tile([128, 512], F32, tag="mm", name="mm")
                    nc.tensor.transpose(wtp[0:64, 0:1], wrow[0:1, :], ident[0:1, 0:1])
                    tw = amp.tile([64, 1], F32, tag=f"tw{s}", name=f"tw{s}")
                    nc.scalar.activation(tw[:, :], wtp[0:64, 0:1], AF.Copy, scale=float(T))
                    for c4 in range(4):
                        sl = slice(c4 * 512, (c4 + 1) * 512)
                        psa = ppm.tile([128, 512], F32, tag="mm", name="mm")
                        _mm(psa[0:64, :], onesr[0:1, 0:64], row_sb[s][0:1, sl], start=True, stop=True)
                        abc = amp.tile([64, 512], F32R, tag=f"abc{s}", name=f"abc{s}")
                        nc.scalar.copy(abc[:, :], psa[0:64, :])
                        ytmp = amp.tile([64, 512], F32, tag=f"ytmp{s}", name=f"ytmp{s}")
                        nc.scalar.activation(ytmp[:, :], yps[:, sl], AF.Identity,
                                             bias=tw[:, 0:1], scale=float(T))
                        nc.vector.tensor_tensor(ytmp[:, :], ytmp[:, :], abc[:, :], ALU.mult)
                        for grp in range(2):
                            nc.sync.dma_start(out=a2a_in[grp * 4 + c4, s * 64:(s + 1) * 64, :],
                                              in_=ytmp[:, :])

            # ---------------- phase 4: AllToAll ----------------
            nc.gpsimd.collective_compute(
                "AllToAll", ALU.bypass,
                replica_groups=[list(range(N_CORES))],
                ins=[a2a_in.opt()],
                outs=[a2a_out.opt()],
            )

            # ---------------- phase 5: proj + LN2 + MLP ----------------
            with tc.tile_pool(name="tail", bufs=1) as tp:
                wproj = [[tp.tile([128, 128], F32R, tag=f"wp{h}{ec}", name=f"wp{h}{ec}") for ec in range(3)] for h in range(H)]
                bproj = tp.tile([128, 3], F32, tag="bproj", name="bproj")
                wf = [[tp.tile([128, 128], F32R, tag=f"wf{jc}{kc}", name=f"wf{jc}{kc}") for kc in range(3)] for jc in range(12)]
                nwft = tp.tile([1, 1536], F32R, tag="nwft", name="nwft")
                ns2f = tp.tile([1, 1536], F32R, tag="ns2f", name="ns2f")
                c2b = tp.tile([128, 12], F32, tag="c2b", name="c2b")
                wf2 = [[tp.tile([128, 128], F32R, tag=f"w2{ec}{kc}", name=f"w2{ec}{kc}") for kc in range(12)] for ec in range(3)]
                bfc2 = tp.tile([128, 3], F32, tag="bfc2", name="bfc2")
                for h in range(H):
                    for ec in range(3):
                        nc.sync.dma_start(out=wproj[h][ec][:, :], in_=wproj_d[h, ec, :, :])
                nc.sync.dma_start(out=bproj[:, :], in_=bproj_d[:, :])
                for jc in range(12):
                    for kc in range(3):
                        nc.sync.dma_start(out=wf[jc][kc][:, :], in_=wf_d[jc, kc, :, :])
                nc.sync.dma_start(out=nwft[:, :], in_=nwft_d[:, :])
                nc.sync.dma_start(out=ns2f[:, :], in_=ns2f_d[:, :])
                nc.sync.dma_start(out=c2b[:, :], in_=c2b_d[:, :])
                for ec in range(3):
                    for kc in range(12):
                        nc.sync.dma_start(out=wf2[ec][kc][:, :], in_=wf2_d[ec, kc, :, :])
                nc.sync.dma_start(out=bfc2[:, :], in_=bfc2_d[:, :])

                stk = [tp.tile([128, 512], F32R, tag=f"stk{h}", name=f"stk{h}") for h in range(H)]
                for h in range(H):
                    c0, s0 = UNIT_SLOT[h]
                    c1_, s1_ = UNIT_SLOT[H + h]
                    nc.sync.dma_start(out=stk[h][0:64, :].bitcast(F32),
                                      in_=a2a_out[c0, s0 * 64:(s0 + 1) * 64, :])
                    nc.sync.dma_start(out=stk[h][64:128, :].bitcast(F32),
                                      in_=a2a_out[c1_, s1_ * 64:(s1_ + 1) * 64, :])

                hT = [tp.tile([128, 512], F32R, tag=f"ht{ec}", name=f"ht{ec}") for ec in range(3)]
                for ec in range(3):
                    ps = ppm.tile([128, 512], F32, tag="mm", name="mm")
                    for h in range(H):
                        _mm(ps[:, :], wproj[h][ec][:, :], stk[h][:, :],
                            start=(h == 0), stop=(h == H - 1))
                    nc.scalar.activation(hT[ec][:, :], ps[:, :], AF.Identity,
                                         bias=bproj[:, ec:ec + 1], scale=1.0)

                # LN2 stats; FC matmuls run on raw hT and get rstd-scaled afterward,
                # so the stats chain overlaps the matmul stream.
                mu2ps = ppm.tile([1, 512], F32, tag="mm", name="mm")
                for ec in range(3):
                    _mm(mu2ps[0:1, :], onesc[:, :], hT[ec][:, :], start=(ec == 0), stop=(ec == 2))
                mu2r = tp.tile([1, 512], F32R, tag="mu2r", name="mu2r")
                nc.scalar.activation(mu2r[0:1, :], mu2ps[0:1, :], AF.Identity,
                                     bias=sbias[0:1, 0:1], scale=1.0 / CP1)
                bneg2 = tp.tile([1, 512], F32R, tag="bneg2", name="bneg2")
                nc.vector.tensor_scalar(bneg2[0:1, :], mu2r[0:1, :], tcol[0:1, 0:1],
                                        None, ALU.subtract)
                scr2 = tp.tile([128, 512], F32R, tag="scr2", name="scr2")
                msq2ps = ppm.tile([1, 512], F32, tag="mm", name="mm")
                for ec in range(3):
                    nc.scalar.square(scr2[:, :], hT[ec][:, :])
                    _mm(msq2ps[0:1, :], onesc[:, :], scr2[:, :], start=(ec == 0), stop=(ec == 2))
                msq2r = tp.tile([1, 512], F32, tag="msq2r", name="msq2r")
                nc.scalar.activation(msq2r[0:1, :], msq2ps[0:1, :], AF.Identity,
                                     bias=sbias[0:1, 1:2], scale=1.0 / CP1)
                v2r = tp.tile([1, 512], F32, tag="v2r", name="v2r")
                nc.vector.tensor_tensor(v2r[0:1, :], mu2r[0:1, :], mu2r[0:1, :], ALU.mult)
                nc.vector.tensor_tensor(v2r[0:1, :], msq2r[0:1, :], v2r[0:1, :], ALU.subtract)
                nc.scalar.activation(v2r[0:1, :], v2r[0:1, :], AF.Sqrt, bias=epsc[0:1, 0:1])
                r2f = tp.tile([1, 512], F32, tag="r2f", name="r2f")
                nc.vector.reciprocal_approx_fast(out=r2f[0:1, :], in_=v2r[0:1, :])
                rstd2r = tp.tile([1, 512], F32R, tag="rstd2r", name="rstd2r")
                nc.vector.tensor_copy(rstd2r[0:1, :], r2f[0:1, :])
                ps = ppm.tile([128, 512], F32, tag="mm", name="mm")
                _mm(ps[:, :], onesr[:, :], rstd2r[0:1, :], start=True, stop=True)
                rstd2bc = tp.tile([128, 512], F32, tag="rstd2bc", name="rstd2bc")
                nc.scalar.copy(rstd2bc[:, :], ps[:, :])

                mT = [tp.tile([128, 512], F32R, tag=f"mt{jc}", name=f"mt{jc}") for jc in range(12)]
                for jc in range(12):
                    pool, tg = (ppm, "mm") if jc % 2 == 0 else (ppt, "tr")
                    zps = pool.tile([128, 512], F32, tag=tg, name="z")
                    zp = zps[:, :]
                    for kc in range(3):
                        _mm(zp, wf[jc][kc][:, :], hT[kc][:, :], start=(kc == 0), stop=False)
                    _mm(zp, ns2f[0:1, jc * 128:(jc + 1) * 128], mu2r[0:1, :], start=False, stop=False)
                    _mm(zp, nwft[0:1, jc * 128:(jc + 1) * 128], bneg2[0:1, :], start=False, stop=True)
                    zsc = tp.tile([128, 512], F32R, tag=f"zsc{jc % 2}", name=f"zsc{jc % 2}")
                    nc.vector.tensor_tensor(zsc[:, :], zp, rstd2bc[:, :], ALU.mult)
                    nc.scalar.activation(mT[jc][:, :], zsc[:, :], AF.Gelu,
                                         bias=c2b[:, jc:jc + 1], scale=1.0)
                for ec in range(3):
                    ps = ppm.tile([128, 512], F32, tag="mm", name="mm")
                    for kc in range(12):
                        _mm(ps[:, :], wf2[ec][kc][:, :], mT[kc][:, :],
                            start=(kc == 0), stop=(kc == 11))
                    oT = tp.tile([128, 512], F32, tag=f"ot{ec}", name=f"ot{ec}")
                    nc.scalar.activation(oT[:, :], ps[:, :], AF.Identity,
                                         bias=bfc2[:, ec:ec + 1], scale=1.0)
                    nc.sync.dma_start(out=out_d[ec * 128:(ec + 1) * 128, :], in_=oT[:, :])

    nc.compile()
    return nc


def host_prep(inputs):
    x = np.asarray(inputs["x"], np.float32)
    t = float(np.asarray(inputs["t"]).reshape(-1)[0])
    w1 = np.asarray(inputs["ln1_w"], np.float32); b1 = np.asarray(inputs["ln1_b"], np.float32)
    Wa = np.asarray(inputs["attn_w"], np.float32); ba = np.asarray(inputs["attn_b"], np.float32)
    Wp_ = w1[:, None] * Wa
    c1 = b1 @ Wa + ba
    Wa_main, Wa_trow = Wp_[:C], Wp_[C]
    s1 = Wp_[:C].sum(axis=0)
    w2 = np.asarray(inputs["ln2_w"], np.float32); b2 = np.asarray(inputs["ln2_b"], np.float32)
    Wf = np.asarray(inputs["fc_w"], np.float32); bf = np.asarray(inputs["fc_b"], np.float32)
    Wf_p = w2[:, None] * Wf
    c2 = b2 @ Wf + bf
    Wf_main, Wf_trow = Wf_p[:C], Wf_p[C]
    s2f = Wf_p[:C].sum(axis=0)
    Wpj = np.asarray(inputs["proj_w"], np.float32); bpj = np.asarray(inputs["proj_b"], np.float32)
    Wf2 = np.asarray(inputs["fc2_w"], np.float32); bf2 = np.asarray(inputs["fc2_b"], np.float32)

    common = {
        "ident": np.eye(128, dtype=np.float32),
        "onesc": np.ones((128, 1), np.float32),
        "onesr": np.ones((1, 128), np.float32),
        "tcol": np.full((128, 1), t, np.float32),
        "sbias": np.array([[t / CP1, t * t / CP1]], np.float32),
        "epsc": np.full((128, 1), EPS, np.float32),
        "tmlt": np.broadcast_to(
            np.array([float(T) * (T - (it + 1) * 128) for it in range(NT)], np.float32),
            (128, NT)).copy(),
        "bproj": bpj.reshape(3, 128).T.astype(np.float32).copy(),
        "c2b": c2.reshape(12, 128).T.astype(np.float32).copy(),
        "bfc2": bf2.reshape(3, 128).T.astype(np.float32).copy(),
        "nwft": (-Wf_trow)[None, :].astype(np.float32).copy(),
        "ns2f": (-s2f)[None, :].astype(np.float32).copy(),
        "wf": np.stack([np.stack([Wf_main[kc * 128:(kc + 1) * 128, jc * 128:(jc + 1) * 128]
                                  for kc in range(3)]) for jc in range(12)]).astype(np.float32),
        "wf2": np.stack([np.stack([Wf2[kc * 128:(kc + 1) * 128, ec * 128:(ec + 1) * 128]
                                   for kc in range(12)]) for ec in range(3)]).astype(np.float32),
    }

    in_maps = []
    for c in range(N_CORES):
        units = CORE_UNITS[c]
        myb = UNITS[units[0]][0]
        m = dict(common)
        m["xT"] = np.ascontiguousarray(x[myb].T)
        shard_b = c // 4  # batch of the row shard this core finishes (receiver side)
        wproj = np.zeros((H, 3, 128, 128), np.float32)
        for h in range(H):
            for ec in range(3):
                blk = Wpj[h * HD:(h + 1) * HD, ec * 128:(ec + 1) * 128]
                if shard_b == 0:
                    wproj[h, ec, 0:64] = blk
                else:
                    wproj[h, ec, 64:128] = blk
        m["wproj"] = wproj
        wqk = np.zeros((2, 3, 128, 128), np.float32)
        r1qk = np.zeros((1, 512), np.float32)
        c1qkr = np.zeros((1, 256), np.float32)
        wv = np.zeros((3, 128, 128), np.float32)
        r1v = np.zeros((1, 256), np.float32)
        c1vr = np.zeros((1, 128), np.float32)
        for s, u in enumerate(units):
            _, h = UNITS[u]
            cq = slice(h * HD, (h + 1) * HD)
            ck = slice(C + h * HD, C + (h + 1) * HD)
            cv = slice(2 * C + h * HD, 2 * C + (h + 1) * HD)
            for kc in range(3):
                wqk[s, kc, :, 0:64] = Wa_main[kc * 128:(kc + 1) * 128, cq]
                wqk[s, kc, :, 64:128] = Wa_main[kc * 128:(kc + 1) * 128, ck]
                wv[kc, :, s * 64:(s + 1) * 64] = Wa_main[kc * 128:(kc + 1) * 128, cv]
            base = 2 * s * 128
            r1qk[0, base:base + 64] = -Wa_trow[cq]; r1qk[0, base + 64:base + 128] = -Wa_trow[ck]
            r1qk[0, base + 128:base + 192] = -s1[cq]; r1qk[0, base + 192:base + 256] = -s1[ck]
            r1v[0, s * 64:(s + 1) * 64] = -Wa_trow[cv]
            r1v[0, 128 + s * 64:128 + (s + 1) * 64] = -s1[cv]
            c1qkr[0, s * 128:s * 128 + 64] = c1[cq]; c1qkr[0, s * 128 + 64:s * 128 + 128] = c1[ck]
            c1vr[0, s * 64:(s + 1) * 64] = c1[cv]
        m["wqk"] = wqk; m["r1qk"] = r1qk; m["c1qkr"] = c1qkr
        m["wv"] = wv; m["r1v"] = r1v; m["c1vr"] = c1vr
        in_maps.append(m)
    return in_maps


def kernel(**inputs):
    if "nc" not in _COMPILED:
        _COMPILED["nc"] = build_program()
    nc = _COMPILED["nc"]
    in_maps = host_prep(inputs)
    res = run_bass_kernel_spmd(nc, in_maps, list(range(N_CORES)))
    out = np.zeros((B, T, C), np.float32)
    for c in range(N_CORES):
        oT = res.results[c]["oT"]
        b, t0 = c // 4, (c % 4) * 512
        out[b, t0:t0 + 512, :] = oT.T
    return out



# revision 52
# speedup vs baseline: 1.6372x; 1.6372x over previous
"""Trainium2 Bass kernel for nn_Block_87428354277599 (sinkhorn-attention transformer block).

Self-contained: hardcodes shapes/sharding. kernel(**inputs) -> (2, 2048, 384) f32.

Sharding (8 cores, SPMD):
- 12 (batch, head) units padded to 16 slots: every core runs 2 attention slots
  (cores 4-7's slot 1 gets zero weights; its junk output is never consumed).
- LN1/LN2 are folded into the QKV / MLP matmuls via host-precomputed weight folds
  plus rank-1 corrections (mu and t-column terms) accumulated on the PE.
- Sinkhorn on the row-softmaxed causal attention == multiplicative matrix scaling
  of S = exp(P). S-1 is lower-triangular, so only the lower triangle (S' = S-1)
  is stored SBUF-resident in both layouts (S' f32, S'^T bf16); the all-ones part
  of S becomes global-sum corrections (kept f32). All matvecs run on the PE.
- y^T slices are exchanged with one AllToAll (each sender duplicates its slices
  into both batch shard groups; receivers mask the wrong batch via zeroed halves
  of the duplicated proj weights). proj+LN2+MLP run row-sharded (512 rows/core).
"""

import numpy as np
import ml_dtypes

import concourse.bacc as bacc
import concourse.mybir as mybir
from concourse.tile import TileContext
from concourse.bass_utils import run_bass_kernel_spmd

F32 = mybir.dt.float32
BF16 = mybir.dt.bfloat16
F32R = mybir.dt.float32r
AF = mybir.ActivationFunctionType
ALU = mybir.AluOpType
AXX = mybir.AxisListType.X

B, T, C, H, HD = 2, 2048, 384, 6, 64
CP1 = C + 1
N_CORES = 8
NT = T // 128  # 16
EPS = 1e-5
UNITS = [(u // H, u % H) for u in range(2 * H)]  # 12 real units
CORE_UNITS = {0: [0, 1], 1: [2, 3], 2: [4, 5], 3: [6, 7], 4: [8], 5: [9], 6: [10], 7: [11]}
UNIT_SLOT = {}
for _c, _us in CORE_UNITS.items():
    for _s, _u in enumerate(_us):
        UNIT_SLOT[_u] = (_c, _s)

_COMPILED = {}


def build_program():
    nc = bacc.Bacc(trn_type="TRN2", num_devices=N_CORES)

    def _mm(out, lhsT, rhs, start, stop):
        nc.tensor.matmul(out, lhsT, rhs, start=start, stop=stop)

    _mmb = _mm

    def din(name, shape, dt=F32):
        return nc.dram_tensor(name, list(shape), dt, kind="ExternalInput")

    xT_d = din("xT", (C, T), F32R)
    wqk_d = din("wqk", (2, 3, 128, 128), F32R)
    wv_d = din("wv", (3, 128, 128), F32R)
    r1qk_d = din("r1qk", (1, 512), F32R)
    r1v_d = din("r1v", (1, 256), F32R)
    c1qkr_d = din("c1qkr", (1, 256), F32R)
    c1vr_d = din("c1vr", (1, 128), F32R)
    ident_d = din("ident", (128, 128))
    onesc_d = din("onesc", (128, 1), F32R)
    onesr_d = din("onesr", (1, 128), F32R)
    tcol_d = din("tcol", (128, 1))
    sbias_d = din("sbias", (1, 2))
    epsc_d = din("epsc", (128, 1))
    tmlt_d = din("tmlt", (128, NT))
    wproj_d = din("wproj", (H, 3, 128, 128), F32R)
    bproj_d = din("bproj", (128, 3))
    wf_d = din("wf", (12, 3, 128, 128), F32R)
    nwft_d = din("nwft", (1, 1536), F32R)
    ns2f_d = din("ns2f", (1, 1536), F32R)
    c2b_d = din("c2b", (128, 12))
    wf2_d = din("wf2", (3, 12, 128, 128), F32R)
    bfc2_d = din("bfc2", (128, 3))
    out_d = nc.dram_tensor("oT", [C, 512], F32, kind="ExternalOutput")

    with TileContext(nc) as tc, nc.allow_low_precision(reason="f32r-typed intermediates (same bits as f32)"):
        with (
            tc.tile_pool(name="const", bufs=1) as cpool,
            tc.tile_pool(name="dram", bufs=1, space="DRAM") as dpool,
            tc.tile_pool(name="ps_wide", bufs=1, space="PSUM") as ppw,
            tc.tile_pool(name="ps_mm", bufs=2, space="PSUM") as ppm,
            tc.tile_pool(name="ps_tr", bufs=2, space="PSUM") as ppt,
            tc.tile_pool(name="qk", bufs=1) as qkp,
        ):
            a2a_in = dpool.tile([8, 128, 512], F32, name="a2a_in")
            a2a_out = dpool.tile([8, 128, 512], F32, name="a2a_out")
            bounce = [dpool.tile([1, T], F32R, name=f"bounce{s}") for s in range(2)]
            bnc_pview = [bounce[s][:, :].rearrange("a (f p) -> (a p) f", p=128) for s in range(2)]

            ident = cpool.tile([128, 128], F32, tag="ident", name="ident")
            onesc = cpool.tile([128, 1], F32R, tag="onesc", name="onesc")
            onesr = cpool.tile([1, 128], F32R, tag="onesr", name="onesr")
            tcol = cpool.tile([128, 1], F32, tag="tcol", name="tcol")
            sbias = cpool.tile([1, 2], F32, tag="sbias", name="sbias")
            epsc = cpool.tile([128, 1], F32, tag="epsc", name="epsc")
            nc.sync.dma_start(out=ident[:, :], in_=ident_d[:, :])
            nc.sync.dma_start(out=onesc[:, :], in_=onesc_d[:, :])
            nc.sync.dma_start(out=onesr[:, :], in_=onesr_d[:, :])
            nc.sync.dma_start(out=tcol[:, :], in_=tcol_d[:, :])
            nc.sync.dma_start(out=sbias[:, :], in_=sbias_d[:, :])
            nc.sync.dma_start(out=epsc[:, :], in_=epsc_d[:, :])
            identr = cpool.tile([128, 128], F32R, tag="identr", name="identr")
            nc.scalar.copy(identr[:, :], ident[:, :])
            ident16 = cpool.tile([128, 128], BF16, tag="ident16", name="ident16")
            nc.scalar.copy(ident16[:, :], ident[:, :])
            onescf = cpool.tile([128, 1], F32, tag="onescf", name="onescf")
            onesrf = cpool.tile([1, 128], F32, tag="onesrf", name="onesrf")
            nc.scalar.copy(onescf[:, :], onesc[:, :])
            nc.scalar.copy(onesrf[:, :], onesr[:, :])
            tmlt = cpool.tile([128, NT], F32, tag="tmlt", name="tmlt")
            nc.sync.dma_start(out=tmlt[:, :], in_=tmlt_d[:, :])

            # persistent per-slot activations (base-partition-0 tiles)
            qT = [qkp.tile([64, T], BF16, tag=f"qT{s}", name=f"qT{s}") for s in range(2)]
            kT = [qkp.tile([64, T], BF16, tag=f"kT{s}", name=f"kT{s}") for s in range(2)]
            vrow = [qkp.tile([128, NT * 64], BF16, tag=f"vrow{s}", name=f"vrow{s}") for s in range(2)]

            # ---------------- phase 1+2: stats + QKV (xt-scoped) ----------------
            with tc.tile_pool(name="xt", bufs=1) as xp:
                xT = [xp.tile([128, T], F32R, tag=f"xt{kc}", name=f"xt{kc}") for kc in range(3)]
                for c4 in range(4):
                    for kc in range(3):
                        nc.sync.dma_start(out=xT[kc][:, c4 * 512:(c4 + 1) * 512],
                                          in_=xT_d[kc * 128:(kc + 1) * 128, c4 * 512:(c4 + 1) * 512])
                wqk = [[xp.tile([128, 128], F32R, tag=f"wqk{s}{kc}", name=f"wqk{s}{kc}") for kc in range(3)] for s in range(2)]
                wv = [xp.tile([128, 128], F32R, tag=f"wv{kc}", name=f"wv{kc}") for kc in range(3)]
                r1qk = xp.tile([1, 512], F32R, tag="r1qk", name="r1qk")
                r1v = xp.tile([1, 256], F32R, tag="r1v", name="r1v")
                c1qkr = xp.tile([1, 256], F32R, tag="c1qkr", name="c1qkr")
                c1vr = xp.tile([1, 128], F32R, tag="c1vr", name="c1vr")
                for s in range(2):
                    for kc in range(3):
                        nc.sync.dma_start(out=wqk[s][kc][:, :], in_=wqk_d[s, kc, :, :])
                for kc in range(3):
                    nc.sync.dma_start(out=wv[kc][:, :], in_=wv_d[kc, :, :])
                nc.sync.dma_start(out=r1qk[:, :], in_=r1qk_d[:, :])
                nc.sync.dma_start(out=r1v[:, :], in_=r1v_d[:, :])
                nc.sync.dma_start(out=c1qkr[:, :], in_=c1qkr_d[:, :])
                nc.sync.dma_start(out=c1vr[:, :], in_=c1vr_d[:, :])

                # ---- stats (per 512-token chunk for pipelining) ----
                mu_row = xp.tile([1, T], F32R, tag="mu_row", name="mu_row")
                msq_row = xp.tile([1, T], F32, tag="msq_row", name="msq_row")
                std_row = xp.tile([1, T], F32R, tag="std_row", name="std_row")
                rstdf = xp.tile([1, T], F32, tag="rstdf", name="rstdf")
                rstd_row = xp.tile([1, T], F32R, tag="rstd_row", name="rstd_row")
                bneg_row = xp.tile([1, T], F32R, tag="bneg_row", name="bneg_row")
                rstd_bc = xp.tile([128, T], F32, tag="rstd_bc", name="rstd_bc")
                wide = ppw.tile([128, T], F32, tag="wide", name="wide")
                for c4 in range(4):
                    sl = slice(c4 * 512, (c4 + 1) * 512)
                    for kc in range(3):
                        _mm(wide[0:1, sl], onesc[:, :], xT[kc][:, sl],
                            start=(kc == 0), stop=(kc == 2))
                    nc.scalar.activation(mu_row[0:1, sl], wide[0:1, sl],
                                         AF.Identity, bias=sbias[0:1, 0:1], scale=1.0 / CP1)
                    ps = ppm.tile([1, 512], F32, tag="mm", name="mm")
                    for kc in range(3):
                        sq = xp.tile([128, 512], F32R, tag=f"scr{kc % 2}", name="scr")
                        nc.vector.tensor_tensor(sq[:, :], xT[kc][:, sl], xT[kc][:, sl], ALU.mult)
                        _mm(ps[0:1, :], onesc[:, :], sq[:, :], start=(kc == 0), stop=(kc == 2))
                    nc.scalar.activation(msq_row[0:1, sl], ps[0:1, :],
                                         AF.Identity, bias=sbias[0:1, 1:2], scale=1.0 / CP1)
                    nc.vector.tensor_tensor(std_row[0:1, sl], mu_row[0:1, sl], mu_row[0:1, sl], ALU.mult)
                    nc.vector.tensor_tensor(std_row[0:1, sl], msq_row[0:1, sl], std_row[0:1, sl], ALU.subtract)
                    nc.scalar.activation(std_row[0:1, sl], std_row[0:1, sl], AF.Sqrt, bias=epsc[0:1, 0:1])
                    nc.vector.reciprocal_approx_fast(out=rstdf[0:1, sl], in_=std_row[0:1, sl].bitcast(F32))
                    nc.vector.tensor_copy(rstd_row[0:1, sl], rstdf[0:1, sl])
                    nc.vector.tensor_scalar(bneg_row[0:1, sl], mu_row[0:1, sl], tcol[0:1, 0:1],
                                            None, ALU.subtract)
                    ps2 = ppm.tile([128, 512], F32, tag="mm", name="mm")
                    _mm(ps2[:, :], onesr[:, :], rstd_row[0:1, sl], start=True, stop=True)
                    nc.scalar.copy(rstd_bc[:, sl], ps2[:, :])

                # ---- QKV matmuls: q|k packed 128-wide, bf16 staging, DMA split ----
                v_c = xp.tile([128, T], F32R, tag="v_c", name="v_c")
                qk_cb = [xp.tile([128, T], BF16, tag=f"qk_cb{s}", name=f"qk_cb{s}") for s in range(2)]

                def qkv_mat(dst, lhsT_chunks, r1_trow, r1_s1, c1row):
                    for c4 in range(4):
                        sl = slice(c4 * 512, (c4 + 1) * 512)
                        ps = ppm.tile([128, 512], F32, tag="mm", name="mm")
                        for kc in range(3):
                            _mm(ps[:, :], lhsT_chunks[kc][:, :], xT[kc][:, sl],
                                start=(kc == 0), stop=False)
                        _mm(ps[:, :], r1_trow, bneg_row[0:1, sl], start=False, stop=False)
                        _mm(ps[:, :], r1_s1, mu_row[0:1, sl], start=False, stop=False)
                        # + c1 (x-independent bias) pre-divided by rstd: c1 (x) std
                        _mm(ps[:, :], c1row, std_row[0:1, sl], start=False, stop=True)
                        nc.vector.tensor_tensor(dst[:, sl], ps[:, :], rstd_bc[:, sl], ALU.mult)

                for s in range(2):
                    b0 = 2 * s * 128
                    qkv_mat(qk_cb[s], wqk[s], r1qk[0:1, b0:b0 + 128],
                            r1qk[0:1, b0 + 128:b0 + 256], c1qkr[0:1, s * 128:(s + 1) * 128])
                qkv_mat(v_c, wv, r1v[0:1, 0:128], r1v[0:1, 128:256], c1vr[0:1, 0:128])
                for s in range(2):
                    nc.sync.dma_start(out=qT[s][:, :], in_=qk_cb[s][0:64, :])
                    nc.sync.dma_start(out=kT[s][:, :], in_=qk_cb[s][64:128, :])

                # v -> row-major bf16 via PE transposes
                vA = xp.tile([64, T], F32R, tag="vA", name="vA")
                vB = xp.tile([64, T], F32R, tag="vB", name="vB")
                nc.sync.dma_start(out=vA[:, :], in_=v_c[0:64, :])
                nc.sync.dma_start(out=vB[:, :], in_=v_c[64:128, :])
                for s, vsrc in ((0, vA), (1, vB)):
                    for g0 in range(0, NT, 4):
                        tr = ppt.tile([128, 512], F32R, tag="tr", name="tr")
                        for gi in range(4):
                            jt = g0 + gi
                            nc.tensor.transpose(tr[:, gi * 128:gi * 128 + 64],
                                                vsrc[:, jt * 128:(jt + 1) * 128], identr[0:64, 0:64])
                        for gi in range(4):
                            nc.vector.tensor_copy(vrow[s][:, (g0 + gi) * 64:(g0 + gi + 1) * 64],
                                                  tr[:, gi * 128:gi * 128 + 64])

            # ------- phase 3: attention, both slots interleaved (bf16 triangles) -------
            with (
                tc.tile_pool(name="sp", bufs=1) as spp,
                tc.tile_pool(name="spt", bufs=1) as sptp,
                tc.tile_pool(name="att_misc", bufs=1) as amp,
            ):
                sp = [[spp.tile([128, (it + 1) * 128], BF16, tag=f"sp{s}_{it}", name=f"sp{s}_{it}")
                       for it in range(NT)] for s in range(2)]
                spt = [[sptp.tile([128, (NT - jt) * 128], BF16, tag=f"spt{s}_{jt}", name=f"spt{s}_{jt}")
                        for jt in range(NT)] for s in range(2)]
                e = [[spt[s][NT - 1 - it] for it in range(NT)] for s in range(2)]  # aliases

                zall = [amp.tile([128, NT], F32, tag=f"zall{s}", name=f"zall{s}") for s in range(2)]
                rz = [amp.tile([128, NT], F32, tag=f"rz{s}", name=f"rz{s}") for s in range(2)]
                ssum = [amp.tile([128, NT], F32, tag=f"ssum{s}", name=f"ssum{s}") for s in range(2)]
                apf = [amp.tile([128, NT], F32, tag=f"apf{s}", name=f"apf{s}") for s in range(2)]
                bpf = [amp.tile([128, NT], F32, tag=f"bpf{s}", name=f"bpf{s}") for s in range(2)]
                a16 = [amp.tile([128, NT], BF16, tag=f"a16{s}", name=f"a16{s}") for s in range(2)]
                b16 = [amp.tile([128, NT], BF16, tag=f"b16{s}", name=f"b16{s}") for s in range(2)]
                row_sb = [amp.tile([1, T], F32R, tag=f"row_sb{s}", name=f"row_sb{s}") for s in range(2)]

                # ---- QK^T + exp(qk/8), causal-masked; z via one DVE row reduce ----
                for it in range(NT):
                    L = (it + 1) * 128
                    d0 = it * 128
                    nch = (L + 511) // 512
                    for s in range(2):
                        for c4 in range(nch):
                            lo, hi = c4 * 512, min(L, (c4 + 1) * 512)
                            ps = ppm.tile([128, 512], F32, tag="mm", name="mm")
                            _mm(ps[:, 0:hi - lo], qT[s][:, d0:d0 + 128], kT[s][:, lo:hi],
                                start=True, stop=True)
                            nc.scalar.activation(e[s][it][:, lo:hi], ps[:, 0:hi - lo],
                                                 AF.Exp, scale=0.125)
                        nc.gpsimd.affine_select(out=e[s][it][:, d0:L], in_=e[s][it][:, d0:L],
                                                compare_op=ALU.is_ge, fill=0.0, base=0,
                                                pattern=[[-1, 128]], channel_multiplier=1)
                        nc.vector.tensor_reduce(zall[s][:, it:it + 1], e[s][it][:, 0:L],
                                                axis=AXX, op=ALU.add)
                for s in range(2):
                    nc.vector.reciprocal_approx_fast(out=rz[s][:, :], in_=zall[s][:, :])

                # ---- S' = exp(att)-1; row sums accumulate for free; transposes ride
                # the PE as soon as their source tiles are ready ----
                for it in range(NT):
                    L = (it + 1) * 128
                    for s in range(2):
                        nc.scalar.activation(sp[s][it][:, :], e[s][it][:, 0:L], AF.Exp,
                                             scale=rz[s][:, it:it + 1],
                                             accum_out=ssum[s][:, it:it + 1])
                        nc.vector.tensor_scalar(sp[s][it][:, :], sp[s][it][:, :], -1.0,
                                                None, ALU.add)
                # transpose groups ordered by the last source tile they need
                groups = []
                for s in range(2):
                    for jt in range(NT):
                        nit = NT - jt
                        for g0 in range(0, nit, 4):
                            gn = min(4, nit - g0)
                            groups.append((jt + g0 + gn - 1, s, jt, g0, gn))
                groups.sort()
                for cnt, (_, s, jt, g0, gn) in enumerate(groups):
                    tr = ppt.tile([128, 1024], BF16, tag="tr", name="tr")
                    for gi in range(gn):
                        it = jt + g0 + gi
                        nc.tensor.transpose(tr[:, gi * 128:(gi + 1) * 128],
                                            sp[s][it][:, jt * 128:(jt + 1) * 128],
                                            ident16[:, :])
                    if cnt % 3 == 0:
                        nc.scalar.copy(spt[s][jt][:, g0 * 128:(g0 + gn) * 128], tr[:, 0:gn * 128])
                    else:
                        nc.vector.tensor_copy(spt[s][jt][:, g0 * 128:(g0 + gn) * 128], tr[:, 0:gn * 128])
                # first sinkhorn u-update is free: a1 = 1/(T*(T - L + rowsum(exp)))
                for s in range(2):
                    nc.vector.scalar_tensor_tensor(apf[s][:, :], ssum[s][:, :], float(T),
                                                   tmlt[:, :], ALU.mult, ALU.add)
                    nc.vector.reciprocal_approx_fast(out=apf[s][:, :], in_=apf[s][:, :])
                    nc.vector.tensor_copy(a16[s][:, :], apf[s][:, :])

                def gsum_col(src_p, tag):
                    red = amp.tile([128, 1], F32, tag=f"red{tag}", name=f"red{tag}")
                    nc.vector.tensor_reduce(red[:, :], src_p[:, :], axis=AXX, op=ALU.add)
                    ps1 = ppm.tile([1, 512], F32, tag="mm", name="mm")
                    _mm(ps1[0:1, 0:1], onescf[:, :], red[:, :], start=True, stop=True)
                    ssb = amp.tile([1, 1], F32, tag=f"ssb{tag}", name=f"ssb{tag}")
                    nc.scalar.copy(ssb[0:1, :], ps1[0:1, 0:1])
                    psb = ppm.tile([128, 512], F32, tag="mm", name="mm")
                    _mm(psb[:, 0:1], onesrf[:, :], ssb[0:1, 0:1], start=True, stop=True)
                    bc = amp.tile([128, 1], F32, tag=f"bc{tag}", name=f"bc{tag}")
                    nc.scalar.copy(bc[:, :], psb[:, 0:1])
                    return bc

                # ---- sinkhorn: a1 done; now b1, (a2, b2), (a3, b3) ----
                wide = ppw.tile([128, T], F32, tag="wide", name="wide")
                for itr in range(3):
                    # v-update: b = 1/(T*(sum(a) + S'^T a)), S'^T a via sp row-tiles
                    Acol = [gsum_col(apf[s], f"a{s}") for s in range(2)]
                    for s in range(2):
                        for it in range(NT):
                            L = (it + 1) * 128
                            for c4 in range((L + 511) // 512):
                                lo, hi = c4 * 512, min(L, (c4 + 1) * 512)
                                _mm(wide[32 * s:32 * s + 1, lo:hi], a16[s][:, it:it + 1], sp[s][it][:, lo:hi],
                                    start=(it == c4 * 4), stop=(it == NT - 1))
                        nc.scalar.copy(row_sb[s][0:1, 0:1024], wide[32 * s:32 * s + 1, 0:1024])
                        nc.vector.tensor_copy(row_sb[s][0:1, 1024:T], wide[32 * s:32 * s + 1, 1024:T])
                        nc.sync.dma_start(out=bounce[s][:, :], in_=row_sb[s][0:1, :])
                        nc.sync.dma_start(out=bpf[s][:, :].bitcast(F32R), in_=bnc_pview[s])
                        nc.vector.tensor_scalar(bpf[s][:, :], bpf[s][:, :], Acol[s][:, 0:1],
                                                float(T), ALU.add, ALU.mult)
                        nc.vector.reciprocal_approx_fast(out=bpf[s][:, :], in_=bpf[s][:, :])
                        nc.vector.tensor_copy(b16[s][:, :], bpf[s][:, :])
                    if itr == 2:
                        break
                    # u-update: a = 1/(T*(sum(b) + S' b)), S' b via spt col-tiles
                    Bcol = [gsum_col(bpf[s], f"b{s}") for s in range(2)]
                    for s in range(2):
                        for jt in range(NT):
                            j0 = jt * 128
                            for c4 in range(4):
                                lo, hi = c4 * 512, (c4 + 1) * 512
                                if hi <= j0:
                                    continue
                                slo = max(lo, j0)
                                _mmb(wide[32 * s:32 * s + 1, slo:hi], b16[s][:, jt:jt + 1],
                                     spt[s][jt][:, slo - j0:hi - j0],
                                     start=(jt == 0), stop=(jt == min(NT - 1, 4 * c4 + 3)))
                        nc.scalar.copy(row_sb[s][0:1, 0:1024], wide[32 * s:32 * s + 1, 0:1024])
                        nc.vector.tensor_copy(row_sb[s][0:1, 1024:T], wide[32 * s:32 * s + 1, 1024:T])
                        nc.sync.dma_start(out=bounce[s][:, :], in_=row_sb[s][0:1, :])
                        nc.sync.dma_start(out=apf[s][:, :].bitcast(F32R), in_=bnc_pview[s])
                        nc.vector.tensor_scalar(apf[s][:, :], apf[s][:, :], Bcol[s][:, 0:1],
                                                float(T), ALU.add, ALU.mult)
                        nc.vector.reciprocal_approx_fast(out=apf[s][:, :], in_=apf[s][:, :])
                        nc.vector.tensor_copy(a16[s][:, :], apf[s][:, :])

                # ---- y^T = T*a ∘ (S' @ (b∘V) + colsum(b∘V)) ----
                for s in range(2):
                    nc.sync.dma_start(out=bnc_pview[s], in_=apf[s][:, :].bitcast(F32R))
                    nc.sync.dma_start(out=row_sb[s][0:1, :], in_=bounce[s][:, :])
                for s in range(2):
                    yps = wide[64:128, :]
                    wcps = ppm.tile([128, 512], F32, tag="mm", name="mm")
                    for jt in range(NT):
                        j0 = jt * 128
                        bv = amp.tile([128, 64], F32, tag=f"bv{s}_{jt % 2}", name=f"bv{s}")
                        nc.vector.tensor_scalar(bv[:, :], vrow[s][:, jt * 64:(jt + 1) * 64],
                                                bpf[s][:, jt:jt + 1], None, ALU.mult)
                        bvh = amp.tile([128, 64], BF16, tag=f"bvh{s}_{jt % 2}", name=f"bvh{s}")
                        nc.vector.tensor_copy(bvh[:, :], bv[:, :])
                        for c4 in range(4):
                            lo, hi = c4 * 512, (c4 + 1) * 512
                            if hi <= j0:
                                continue
                            slo = max(lo, j0)
                            _mmb(yps[:, slo:hi], bvh[:, :], spt[s][jt][:, slo - j0:hi - j0],
                                 start=(jt == 0), stop=(jt == min(NT - 1, 4 * c4 + 3)))
                        _mm(wcps[0:1, 0:64], onescf[:, :], bv[:, :],
                            start=(jt == 0), stop=(jt == NT - 1))
                    wrow = amp.tile([1, 64], F32, tag=f"wrow{s}", name=f"wrow{s}")
                    nc.scalar.copy(wrow[0:1, :], wcps[0:1, 0:64])
                    wtp = ppm.tile([128, 512], F32, tag="mm", name="mm")
                    nc.tensor.transpose(wtp[0:64, 0:1], wrow[0:1, :], ident[0:1, 0:1])
                    tw = amp.tile([64, 1], F32, tag=f"tw{s}", name=f"tw{s}")
                    nc.scalar.activation(tw[:, :], wtp[0:64, 0:1], AF.Copy, scale=float(T))
                    for c4 in range(4):
                        sl = slice(c4 * 512, (c4 + 1) * 512)
                        psa = ppm.tile([128, 512], F32, tag="mm", name="mm")
                        _mm(psa[0:64, :], onesr[0:1, 0:64], row_sb[s][0:1, sl], start=True, stop=True)
                        abc = amp.tile([64, 512], F32R, tag=f"abc{s}", name=f"abc{s}")
                        nc.scalar.copy(abc[:, :], psa[0:64, :])
                        ytmp = amp.tile([64, 512], F32, tag=f"ytmp{s}", name=f"ytmp{s}")
                        nc.scalar.activation(ytmp[:, :], yps[:, sl], AF.Identity,
                                             bias=tw[:, 0:1], scale=float(T))
                        nc.vector.tensor_tensor(ytmp[:, :], ytmp[:, :], abc[:, :], ALU.mult)
                        for grp in range(2):
                            nc.sync.dma_start(out=a2a_in[grp * 4 + c4, s * 64:(s + 1) * 64, :],
                                              in_=ytmp[:, :])

            # ---------------- phase 4: AllToAll ----------------
            nc.gpsimd.collective_compute(
                "AllToAll", ALU.bypass,
                replica_groups=[list(range(N_CORES))],
                ins=[a2a_in.opt()],
                outs=[a2a_out.opt()],
            )

            # ---------------- phase 5: proj + LN2 + MLP ----------------
            with tc.tile_pool(name="tail", bufs=1) as tp:
                wproj = [[tp.tile([128, 128], F32R, tag=f"wp{h}{ec}", name=f"wp{h}{ec}") for ec in range(3)] for h in range(H)]
                bproj = tp.tile([128, 3], F32, tag="bproj", name="bproj")
                wf = [[tp.tile([128, 128], F32R, tag=f"wf{jc}{kc}", name=f"wf{jc}{kc}") for kc in range(3)] for jc in range(12)]
                nwft = tp.tile([1, 1536], F32R, tag="nwft", name="nwft")
                ns2f = tp.tile([1, 1536], F32R, tag="ns2f", name="ns2f")
                c2b = tp.tile([128, 12], F32, tag="c2b", name="c2b")
                wf2 = [[tp.tile([128, 128], F32R, tag=f"w2{ec}{kc}", name=f"w2{ec}{kc}") for kc in range(12)] for ec in range(3)]
                bfc2 = tp.tile([128, 3], F32, tag="bfc2", name="bfc2")
                for h in range(H):
                    for ec in range(3):
                        nc.sync.dma_start(out=wproj[h][ec][:, :], in_=wproj_d[h, ec, :, :])
                nc.sync.dma_start(out=bproj[:, :], in_=bproj_d[:, :])
                for jc in range(12):
                    for kc in range(3):
                        nc.sync.dma_start(out=wf[jc][kc][:, :], in_=wf_d[jc, kc, :, :])
                nc.sync.dma_start(out=nwft[:, :], in_=nwft_d[:, :])
                nc.sync.dma_start(out=ns2f[:, :], in_=ns2f_d[:, :])
                nc.sync.dma_start(out=c2b[:, :], in_=c2b_d[:, :])
                for ec in range(3):
                    for kc in range(12):
                        nc.sync.dma_start(out=wf2[ec][kc][:, :], in_=wf2_d[ec, kc, :, :])
                nc.sync.dma_start(out=bfc2[:, :], in_=bfc2_d[:, :])

                stk = [tp.tile([128, 512], F32R, tag=f"stk{h}", name=f"stk{h}") for h in range(H)]
                for h in range(H):
                    c0, s0 = UNIT_SLOT[h]
                    c1_, s1_ = UNIT_SLOT[H + h]
                    nc.sync.dma_start(out=stk[h][0:64, :].bitcast(F32),
                                      in_=a2a_out[c0, s0 * 64:(s0 + 1) * 64, :])
                    nc.sync.dma_start(out=stk[h][64:128, :].bitcast(F32),
                                      in_=a2a_out[c1_, s1_ * 64:(s1_ + 1) * 64, :])

                hT = [tp.tile([128, 512], F32R, tag=f"ht{ec}", name=f"ht{ec}") for ec in range(3)]
                for ec in range(3):
                    ps = ppm.tile([128, 512], F32, tag="mm", name="mm")
                    for h in range(H):
                        _mm(ps[:, :], wproj[h][ec][:, :], stk[h][:, :],
                            start=(h == 0), stop=(h == H - 1))
                    nc.scalar.activation(hT[ec][:, :], ps[:, :], AF.Identity,
                                         bias=bproj[:, ec:ec + 1], scale=1.0)

                # LN2 stats; FC matmuls run on raw hT and get rstd-scaled afterward,
                # so the stats chain overlaps the matmul stream.
                mu2ps = ppm.tile([1, 512], F32, tag="mm", name="mm")
                for ec in range(3):
                    _mm(mu2ps[0:1, :], onesc[:, :], hT[ec][:, :], start=(ec == 0), stop=(ec == 2))
                mu2r = tp.tile([1, 512], F32R, tag="mu2r", name="mu2r")
                nc.scalar.activation(mu2r[0:1, :], mu2ps[0:1, :], AF.Identity,
                                     bias=sbias[0:1, 0:1], scale=1.0 / CP1)
                bneg2 = tp.tile([1, 512], F32R, tag="bneg2", name="bneg2")
                nc.vector.tensor_scalar(bneg2[0:1, :], mu2r[0:1, :], tcol[0:1, 0:1],
                                        None, ALU.subtract)
                scr2 = tp.tile([128, 512], F32R, tag="scr2", name="scr2")
                msq2ps = ppm.tile([1, 512], F32, tag="mm", name="mm")
                for ec in range(3):
                    nc.scalar.square(scr2[:, :], hT[ec][:, :])
                    _mm(msq2ps[0:1, :], onesc[:, :], scr2[:, :], start=(ec == 0), stop=(ec == 2))
                msq2r = tp.tile([1, 512], F32, tag="msq2r", name="msq2r")
                nc.scalar.activation(msq2r[0:1, :], msq2ps[0:1, :], AF.Identity,
                                     bias=sbias[0:1, 1:2], scale=1.0 / CP1)
                v2r = tp.tile([1, 512], F32, tag="v2r", name="v2r")
                nc.vector.tensor_tensor(v2r[0:1, :], mu2r[0:1, :], mu2r[0:1, :], ALU.mult)
                nc.vector.tensor_tensor(v2r[0:1, :], msq2r[0:1, :], v2r[0:1, :], ALU.subtract)
                nc.scalar.activation(v2r[0:1, :], v2r[0:1, :], AF.Sqrt, bias=epsc[0:1, 0:1])
                r2f = tp.tile([1, 512], F32, tag="r2f", name="r2f")
                nc.vector.reciprocal_approx_fast(out=r2f[0:1, :], in_=v2r[0:1, :])
                rstd2r = tp.tile([1, 512], F32R, tag="rstd2r", name="rstd2r")
                nc.vector.tensor_copy(rstd2r[0:1, :], r2f[0:1, :])
                ps = ppm.tile([128, 512], F32, tag="mm", name="mm")
                _mm(ps[:, :], onesr[:, :], rstd2r[0:1, :], start=True, stop=True)
                rstd2bc = tp.tile([128, 512], F32, tag="rstd2bc", name="rstd2bc")
                nc.scalar.copy(rstd2bc[:, :], ps[:, :])

                mT = [tp.tile([128, 512], F32R, tag=f"mt{jc}", name=f"mt{jc}") for jc in range(12)]
                for jc in range(12):
                    pool, tg = (ppm, "mm") if jc % 2 == 0 else (ppt, "tr")
                    zps = pool.tile([128, 512], F32, tag=tg, name="z")
                    zp = zps[:, :]
                    for kc in range(3):
                        _mm(zp, wf[jc][kc][:, :], hT[kc][:, :], start=(kc == 0), stop=False)
                    _mm(zp, ns2f[0:1, jc * 128:(jc + 1) * 128], mu2r[0:1, :], start=False, stop=False)
                    _mm(zp, nwft[0:1, jc * 128:(jc + 1) * 128], bneg2[0:1, :], start=False, stop=True)
                    zsc = tp.tile([128, 512], F32R, tag=f"zsc{jc % 2}", name=f"zsc{jc % 2}")
                    nc.vector.tensor_tensor(zsc[:, :], zp, rstd2bc[:, :], ALU.mult)
                    nc.scalar.activation(mT[jc][:, :], zsc[:, :], AF.Gelu,
                                         bias=c2b[:, jc:jc + 1], scale=1.0)
                for ec in range(3):
                    ps = ppm.tile([128, 512], F32, tag="mm", name="mm")
                    for kc in range(12):
                        _mm(ps[:, :], wf2[ec][kc][:, :], mT[kc][:, :],
                            start=(kc == 0), stop=(kc == 11))
                    oT = tp.tile([128, 512], F32, tag=f"ot{ec}", name=f"ot{ec}")
                    nc.scalar.activation(oT[:, :], ps[:, :], AF.Identity,
                                         bias=bfc2[:, ec:ec + 1], scale=1.0)
                    nc.sync.dma_start(out=out_d[ec * 128:(ec + 1) * 128, :], in_=oT[:, :])

    nc.compile()
    return nc


def host_prep(inputs):
    x = np.asarray(inputs["x"], np.float32)
    t = float(np.asarray(inputs["t"]).reshape(-1)[0])
    w1 = np.asarray(inputs["ln1_w"], np.float32); b1 = np.asarray(inputs["ln1_b"], np.float32)
    Wa = np.asarray(inputs["attn_w"], np.float32); ba = np.asarray(inputs["attn_b"], np.float32)
    Wp_ = w1[:, None] * Wa
    c1 = b1 @ Wa + ba
    Wa_main, Wa_trow = Wp_[:C], Wp_[C]
    s1 = Wp_[:C].sum(axis=0)
    w2 = np.asarray(inputs["ln2_w"], np.float32); b2 = np.asarray(inputs["ln2_b"], np.float32)
    Wf = np.asarray(inputs["fc_w"], np.float32); bf = np.asarray(inputs["fc_b"], np.float32)
    Wf_p = w2[:, None] * Wf
    c2 = b2 @ Wf + bf
    Wf_main, Wf_trow = Wf_p[:C], Wf_p[C]
    s2f = Wf_p[:C].sum(axis=0)
    Wpj = np.asarray(inputs["proj_w"], np.float32); bpj = np.asarray(inputs["proj_b"], np.float32)
    Wf2 = np.asarray(inputs["fc2_w"], np.float32); bf2 = np.asarray(inputs["fc2_b"], np.float32)

    common = {
        "ident": np.eye(128, dtype=np.float32),
        "onesc": np.ones((128, 1), np.float32),
        "onesr": np.ones((1, 128), np.float32),
        "tcol": np.full((128, 1), t, np.float32),
        "sbias": np.array([[t / CP1, t * t / CP1]], np.float32),
        "epsc": np.full((128, 1), EPS, np.float32),
        "tmlt": np.broadcast_to(
            np.array([float(T) * (T - (it + 1) * 128) for it in range(NT)], np.float32),
            (128, NT)).copy(),
        "bproj": bpj.reshape(3, 128).T.astype(np.float32).copy(),
        "c2b": c2.reshape(12, 128).T.astype(np.float32).copy(),
        "bfc2": bf2.reshape(3, 128).T.astype(np.float32).copy(),
        "nwft": (-Wf_trow)[None, :].astype(np.float32).copy(),
        "ns2f": (-s2f)[None, :].astype(np.float32).copy(),
        "wf": np.stack([np.stack([Wf_main[kc * 128:(kc + 1) * 128, jc * 128:(jc + 1) * 128]
                                  for kc in range(3)]) for jc in range(12)]).astype(np.float32),
        "wf2": np.stack([np.stack([Wf2[kc * 128:(kc + 1) * 128, ec * 128:(ec + 1) * 128]
                                   for kc in range(12)]) for ec in range(3)]).astype(np.float32),
    }

    in_maps = []
    for c in range(N_CORES):
        units = CORE_UNITS[c]
        myb = UNITS[units[0]][0]
        m = dict(common)
        m["xT"] = np.ascontiguousarray(x[myb].T)
        shard_b = c // 4  # batch of the row shard this core finishes (receiver side)
        wproj = np.zeros((H, 3, 128, 128), np.float32)
        for h in range(H):
            for ec in range(3):
                blk = Wpj[h * HD:(h + 1) * HD, ec * 128:(ec + 1) * 128]
                if shard_b == 0:
                    wproj[h, ec, 0:64] = blk
                else:
                    wproj[h, ec, 64:128] = blk
        m["wproj"] = wproj
        wqk = np.zeros((2, 3, 128, 128), np.float32)
        r1qk = np.zeros((1, 512), np.float32)
        c1qkr = np.zeros((1, 256), np.float32)
        wv = np.zeros((3, 128, 128), np.float32)
        r1v = np.zeros((1, 256), np.float32)
        c1vr = np.zeros((1, 128), np.float32)
        for s, u in enumerate(units):
            _, h = UNITS[u]
            cq = slice(h * HD, (h + 1) * HD)
            ck = slice(C + h * HD, C + (h + 1) * HD)
            cv = slice(2 * C + h * HD, 2 * C + (h + 1) * HD)
            for kc in range(3):
                wqk[s, kc, :, 0:64] = Wa_main[kc * 128:(kc + 1) * 128, cq]
                wqk[s, kc, :, 64:128] = Wa_main[kc * 128:(kc + 1) * 128, ck]
                wv[kc, :, s * 64:(s + 1) * 64] = Wa_main[kc * 128:(kc + 1) * 128, cv]
            base = 2 * s * 128
            r1qk[0, base:base + 64] = -Wa_trow[cq]; r1qk[0, base + 64:base + 128] = -Wa_trow[ck]
            r1qk[0, base + 128:base + 192] = -s1[cq]; r1qk[0, base + 192:base + 256] = -s1[ck]
            r1v[0, s * 64:(s + 1) * 64] = -Wa_trow[cv]
            r1v[0, 128 + s * 64:128 + (s + 1) * 64] = -s1[cv]
            c1qkr[0, s * 128:s * 128 + 64] = c1[cq]; c1qkr[0, s * 128 + 64:s * 128 + 128] = c1[ck]
            c1vr[0, s * 64:(s + 1) * 64] = c1[cv]
        m["wqk"] = wqk; m["r1qk"] = r1qk; m["c1qkr"] = c1qkr
        m["wv"] = wv; m["r1v"] = r1v; m["c1vr"] = c1vr
        in_maps.append(m)
    return in_maps


def kernel(**inputs):
    if "nc" not in _COMPILED:
        _COMPILED["nc"] = build_program()
    nc = _COMPILED["nc"]
    in_maps = host_prep(inputs)
    res = run_bass_kernel_spmd(nc, in_maps, list(range(N_CORES)))
    out = np.zeros((B, T, C), np.float32)
    for c in range(N_CORES):
        oT = res.results[c]["oT"]
        b, t0 = c // 4, (c % 4) * 512
        out[b, t0:t0 + 512, :] = oT.T
    return out

